# revision 8
# baseline (speedup 1.0000x reference)
"""Trainium2 Bass kernel for nn_MemoryCell (scatter_memory).

Full-input contract: kernel(**inputs) takes the complete (unsharded) numpy
inputs and returns the full [NB*B, H] output.

Math (B == H == 1024, NB == 5, T == 128):
    enc  = features[:, 0, :]                         # [B, H] - only slice used
    h    = states.reshape(NB, H)
    gate = sigmoid(enc @ (h + keys).T)               # [B, NB]
    pre  = (h @ Uw.T + keys @ Vw.T)[:, None, :] + (enc @ Ww.T)[None, :, :]
    cand = where(pre >= 0, pre, prelu_a * pre)
    new[i, b, j] = h[i, j] + gate[j, i] * cand[i, b, j]   # B==H broadcast quirk
    out  = sign(new) with exact zeros -> +1, reshaped [NB*B, H]

Sharding: split the feature/column axis j (H=1024) into 8 shards of 128
(one per core).  Each core needs: full enc (transposed, for the big
enc @ Ww.T matmul over all b), the j-shard rows of Uw/Vw/Ww/enc, and the
tiny h/keys vectors.  Per-core HBM traffic ~6.9 MB vs ~36 MB unsharded.

Per-core layout: j on SBUF partitions (128 = shard size), b on the free
axis.  Matmuls run in split-fp16 precision: every fp32 operand x ships as
an fp16 pair (hi = fp16(x), lo = fp16(x - hi)) and each K-chunk issues
three 1-cycle/row fp16 matmuls (hi*hi + hi*lo + lo*hi, fp32 PSUM accum).
The dropped lo*lo term and the 2^-22 pair residual keep the result within
~1e-6 of the fp32 product - well inside the sign-flip noise floor - while
using ~2.5x less PE time than fp32's double-pumped 4-cycle/row path.

All small operands arrive in ONE packed fp16 [128, 8432] DMA; enc arrives
as hi/lo in 4 half-major [128, 2, 4, 512] DMAs so the b-half-0 tail can
start while half 1 streams in.  gate/huv use the tiny [128,5] operands as
the stationary (cheap weight loads) producing [5,128] results that one PE
transpose flips back to j-on-partitions.  The elementwise tail is, per
(i, b-half):
    DVE: pre2 = (ew + huv_i) * gate_i      (fused tensor_scalar, psum in)
    ACT: o    = Sign(pre2 + h_i) -> int8
matching the reference's floating-point association order exactly.
Outputs ship as int8 signs (4x fewer bytes) and the host re-expands.
"""

import os
import numpy as np

H = 1024
NB = 5
B = 1024
NCORES = 8
JS = H // NCORES          # 128 columns per core
KC = H // 128             # 8 contraction chunks
NHALF = 2                 # b axis processed in halves of 512 (PSUM bank limit)
HB = B // NHALF

# packed fp16 small-input offsets (fp16 elements per partition)
OFF_W = 0                 # w_hi, w_lo
OFF_G = 2048              # g_hi, g_lo
OFF_U = 4096              # u_hi, u_lo
OFF_V = 6144              # v_hi, v_lo
OFF_HK = 8192             # hk_hi, hk_lo
OFF_H = 8272              # h_hi, h_lo
OFF_K = 8352              # k_hi, k_lo
SMALL_F = 8432

_NC_CACHE = {}


def _build_nc(general_prelu: bool):
    from concourse import bacc, mybir
    import concourse.tile as tile
    from concourse.masks import make_identity

    f32 = mybir.dt.float32
    f16 = mybir.dt.float16
    i8 = mybir.dt.int8
    AF = mybir.ActivationFunctionType
    ALU = mybir.AluOpType

    hs_f = NB + (1 if general_prelu else 0)

    nc = bacc.Bacc("TRN2", debug=False, num_devices=NCORES)

    small = nc.dram_tensor("small", [128, SMALL_F], f16, kind="ExternalInput").ap()
    hs32 = nc.dram_tensor("hs32", [128, hs_f], f32, kind="ExternalInput").ap()
    encT = nc.dram_tensor("encT", [NHALF, 2, 128, 8, HB], f16, kind="ExternalInput").ap()
    out = nc.dram_tensor("out", [1, 128, NB, B], i8, kind="ExternalOutput").ap()

    with tile.TileContext(nc) as tc:
        with (
            tc.tile_pool(name="res", bufs=1) as res,
            tc.tile_pool(name="work", bufs=3) as work,
            tc.tile_pool(name="psmall", bufs=1, space="PSUM") as psmall,
            tc.tile_pool(name="pew", bufs=2, space="PSUM") as pew,
        ):
            # ---- input DMAs (all on SyncE, in priority order) ----
            sm = res.tile([128, SMALL_F], f16, name="sm")
            nc.sync.dma_start(sm, small)
            hs_sb = res.tile([128, hs_f], f32, name="hs_sb")
            nc.sync.dma_start(hs_sb, hs32)

            # enc hi/lo in half-major [128, 2, 4, 512] tiles: (b-half, k-group)
            enc_t = {}
            for half in range(NHALF):
                for grp in range(2):
                    e = res.tile([128, 8, HB], f16, name=f"enc_{half}_{grp}",
                                 tag=f"enc_{half}_{grp}")
                    nc.sync.dma_start(e, encT[half, grp])
                    enc_t[(half, grp)] = e

            def pair(off, k, width):
                hi = sm[:, off + k * width:off + (k + 1) * width]
                lo = sm[:, off + KC * width + k * width:
                        off + KC * width + (k + 1) * width]
                return hi, lo

            def w_sl(k):
                return pair(OFF_W, k, JS)

            def g_sl(k):
                return pair(OFF_G, k, JS)

            def u_sl(k):
                return pair(OFF_U, k, JS)

            def v_sl(k):
                return pair(OFF_V, k, JS)

            def hk_sl(k):
                return pair(OFF_HK, k, NB)

            def h_sl(k):
                return pair(OFF_H, k, NB)

            def k_sl(k):
                return pair(OFF_K, k, NB)

            def mm3(psum, stat, mov, k, first, last):
                s_hi, s_lo = stat(k)
                m_hi, m_lo = mov(k)
                nc.tensor.matmul(psum, lhsT=s_hi, rhs=m_hi,
                                 start=first, stop=False)
                nc.tensor.matmul(psum, lhsT=s_hi, rhs=m_lo,
                                 start=False, stop=False)
                nc.tensor.matmul(psum, lhsT=s_lo, rhs=m_hi,
                                 start=False, stop=last)

            # ---- gate / huv as [5, 128] with the tiny operand stationary ----
            psum_gT = psmall.tile([NB, 128], f32, name="psum_gT")
            for k in range(KC):
                mm3(psum_gT, hk_sl, g_sl, k, k == 0, k == KC - 1)
            psum_hT = psmall.tile([NB, 128], f32, name="psum_hT")
            for k in range(KC):
                mm3(psum_hT, h_sl, u_sl, k, k == 0, False)
            for k in range(KC):
                mm3(psum_hT, k_sl, v_sl, k, False, k == KC - 1)

            # transpose [gate;huv] -> [128, .] via the PE
            identity = res.tile([128, 128], f32, name="identity")
            make_identity(nc, identity)
            gh_sb = res.tile([128, 128], f32, name="gh_sb")
            nc.gpsimd.memset(gh_sb, 0.0)
            # compute-engine partition bases must be 32-aligned: gate rows
            # live at partitions 0:5, huv rows at 32:37
            nc.vector.tensor_copy(out=gh_sb[0:NB, :], in_=psum_gT)
            nc.vector.tensor_copy(out=gh_sb[32:32 + NB, :], in_=psum_hT)
            psum_gh = psmall.tile([128, 128], f32, name="psum_gh")
            nc.tensor.transpose(psum_gh, gh_sb, identity)

            gate_sb = res.tile([128, NB], f32, name="gate_sb")
            nc.scalar.activation(gate_sb, psum_gh[:, 0:NB], AF.Sigmoid)
            huv_sb = res.tile([128, NB], f32, name="huv_sb")
            nc.vector.tensor_copy(out=huv_sb, in_=psum_gh[:, 32:32 + NB])
            # bias3 = gate*huv + h_s: folds the per-block offset into the
            # activation bias so the tail is one ScalarE op per (i, half)
            bias3 = res.tile([128, NB], f32, name="bias3")
            nc.vector.tensor_tensor(bias3, gate_sb, huv_sb, ALU.mult)
            nc.vector.tensor_tensor(bias3, bias3, hs_sb[:, 0:NB], ALU.add)

            # ---- ew = enc @ Ww[js].T (j on partitions, b on free) + tail ----
            o_all = work.tile([128, NB, B], i8, name="o_all", tag="o_all",
                              bufs=1)
            for half in range(NHALF):
                pew_t = pew.tile([128, HB], f32, name="pew_t", tag="ew")
                for k in range(KC):
                    w_hi, w_lo = w_sl(k)
                    et = enc_t[(half, k // 4)]
                    e_hi = et[:, (k % 4) * 2, :]
                    e_lo = et[:, (k % 4) * 2 + 1, :]
                    nc.tensor.matmul(pew_t, lhsT=w_hi, rhs=e_hi,
                                     start=(k == 0), stop=False)
                    nc.tensor.matmul(pew_t, lhsT=w_hi, rhs=e_lo,
                                     start=False, stop=False)
                    nc.tensor.matmul(pew_t, lhsT=w_lo, rhs=e_hi,
                                     start=False, stop=(k == KC - 1))
                for i in range(NB):
                    if general_prelu:
                        a_col = hs_sb[:, NB:NB + 1]
                        pre = work.tile([128, HB], f32, name="pre", tag="pre")
                        nc.vector.tensor_scalar_add(pre, pew_t, huv_sb[:, i:i + 1])
                        mx = work.tile([128, HB], f32, name="mx", tag="mx")
                        nc.vector.tensor_scalar_max(mx, pre, 0.0)
                        mn = work.tile([128, HB], f32, name="mn", tag="mn")
                        nc.vector.tensor_scalar_min(mn, pre, 0.0)
                        cand = work.tile([128, HB], f32, name="cand", tag="cand")
                        nc.vector.scalar_tensor_tensor(
                            cand, in0=mn, scalar=a_col, in1=mx,
                            op0=ALU.mult, op1=ALU.add)
                        nc.scalar.activation(
                            o_all[:, i, half * HB:(half + 1) * HB], cand,
                            AF.Sign, bias=hs_sb[:, i:i + 1],
                            scale=gate_sb[:, i:i + 1])
                    else:
                        # o = Sign(ew*gate_i + (gate_i*huv_i + h_i)), one ACT op
                        nc.scalar.activation(
                            o_all[:, i, half * HB:(half + 1) * HB], pew_t,
                            AF.Sign, bias=bias3[:, i:i + 1],
                            scale=gate_sb[:, i:i + 1])
            nc.gpsimd.dma_start(out[0], o_all)

    nc.compile()
    return nc


def _get_nc(general_prelu: bool):
    nc = _NC_CACHE.get(general_prelu)
    if nc is None:
        nc = _build_nc(general_prelu)
        _NC_CACHE[general_prelu] = nc
    return nc


def _c32(a):
    return np.ascontiguousarray(a, dtype=np.float32)


def _packT(mat_t):
    # [H, F] (k-major rows) -> [128, KC*F]: row p holds blocks k of F values
    F = mat_t.shape[1]
    return mat_t.reshape(KC, 128, F).transpose(1, 0, 2).reshape(128, KC * F)


def _split16(a):
    # fp32 -> (hi, lo) fp16 pair with hi + lo == a to ~2^-22 relative
    hi = a.astype(np.float16)
    lo = (a - hi.astype(np.float32)).astype(np.float16)
    return hi, lo


def kernel(features, states, Uw, Vw, Ww, keys, prelu_a):
    from concourse import bass_utils

    features = np.asarray(features)
    states = np.asarray(states, dtype=np.float32)
    Uw = np.asarray(Uw, dtype=np.float32)
    Vw = np.asarray(Vw, dtype=np.float32)
    Ww = np.asarray(Ww, dtype=np.float32)
    keys = np.asarray(keys, dtype=np.float32)
    prelu_a = np.asarray(prelu_a, dtype=np.float32)

    enc = np.ascontiguousarray(features[:, 0, :], dtype=np.float32)  # [B, H]
    h = states.reshape(NB, H)
    hk = h + keys

    general_prelu = not np.all(prelu_a == 1.0)
    nc = _get_nc(general_prelu)

    enc_hi, enc_lo = _split16(_c32(enc.T))
    # [KC,2,128,B] -> tile layout [half, grp, p, (k_local,t), b-half]
    encP = np.stack([enc_hi.reshape(KC, 128, B), enc_lo.reshape(KC, 128, B)],
                    axis=1)
    encP = encP.reshape(2, 4, 2, 128, NHALF, HB)          # grp,kl,t,p,half,b
    encP = np.ascontiguousarray(encP.transpose(4, 0, 3, 1, 2, 5)
                                .reshape(NHALF, 2, 128, 8, HB))
    hkP = _split16(_packT(_c32(hk.T)))
    hP = _split16(_packT(_c32(h.T)))
    kP = _split16(_packT(_c32(keys.T)))

    in_maps = []
    for c in range(NCORES):
        js = slice(c * JS, (c + 1) * JS)
        parts = []
        for mat in (Ww, None, Uw, Vw):  # None slot = g (enc rows)
            src = enc[js].T if mat is None else mat[js].T
            hi, lo = _split16(_packT(_c32(src)))
            parts += [hi, lo]
        parts += [hkP[0], hkP[1], hP[0], hP[1], kP[0], kP[1]]
        hs_parts = [_c32(h[:, js].T)]
        if general_prelu:
            hs_parts.append(_c32(prelu_a[js].reshape(128, 1)))
        in_maps.append({
            "small": np.ascontiguousarray(np.concatenate(parts, axis=1),
                                          dtype=np.float16),
            "hs32": np.ascontiguousarray(np.concatenate(hs_parts, axis=1),
                                         dtype=np.float32),
            "encT": encP,
        })

    trace = bool(int(os.environ.get("KERNEL_TRACE", "0")))
    res = bass_utils.run_bass_kernel_spmd(
        nc, in_maps, core_ids=list(range(NCORES)), trace=trace)
    kernel.last_result = res

    one = np.float32(1.0)
    neg = np.float32(-1.0)
    full = np.empty((NB, B, H), dtype=np.float32)
    view = full.reshape(NB, B, NCORES, JS)
    for c in range(NCORES):
        oc = res.results[c]["out"][0]  # int8 [128, NB, B]
        # int8 sign >= 0 -> +1 (exact zeros map to +1, as in the reference)
        view[:, :, c, :] = np.where(oc.transpose(1, 2, 0) >= 0, one, neg)
    return full.reshape(NB * B, H)
